# revision 2
# baseline (speedup 1.0000x reference)
"""Trainium2 Bass kernel for nn_DEAttention_Module (dense channel-attention), v3.

Per batch b, X = x[b] as (C=512, N=4096), M = Wq^T Wk, Xk = X[:, 512k:512(k+1)]:
    energy = sum_k Xk^T M Xk ; attn = softmax(energy)
    y_k = gamma * (Wv Xk) attn^T + Xk

Schedule (per core = one batch):
  * Energy path in float32r (1 cycle/row at moving>=256).  x and M^T are
    pre-rounded to f32r bits on the host and DMA'd straight into f32r SBUF
    (walrus accepts DMA-produced f32r when the DRAM tensor is f32r-typed).
  * V path in fp8e4 DoubleRow (2 contraction tiles per matmul, 0.5 c/row)
    with hi/lo error compensation (numpy-validated end-to-end 1.7e-3 max
    rel err vs the 2e-2 gate):
      V1T_k = Xk^T (32 Wv^T)      3 terms: x8.w8 + x8.we8 + xe8.w8
      vq    = fp8_2term(gamma/32 * V1T_k)           (ACT copy + DVE stt)
      y_k   = vq^T aq (3 terms) + I.(x8+xe8)        accumulated in PSUM
    where aq = fp8_2term(attn^T) from bf16 PE transposes.  The residual
    rides through the PE as a DoubleRow identity matmul, so the finished
    y chunk is DMA'd STRAIGHT FROM PSUM -- no copy-out op.
  * Hi/lo pairs live interleaved in one tile ([P, T, 2, ...]) so a (hi,lo)
    pair is a single [P,2,*] AP slice = one DoubleRow operand.
  * gamma/32 is baked into an ACT immediate (nc cached per gamma value).
  * Sharding: data-parallel over batch B=8 across the 8 cores.
"""
import sys
from contextlib import ExitStack

sys.path.insert(0, "/opt/trn_rl_repo")

import numpy as np

import concourse.bacc as bacc
import concourse.bass as bass
import concourse.tile as tile
from concourse import mybir
from concourse.bass_utils import run_bass_kernel_spmd
from concourse.masks import make_identity

f32 = mybir.dt.float32
f32r = mybir.dt.float32r
bf16 = mybir.dt.bfloat16
fp8 = mybir.dt.float8e4

P = 128
T = 4     # channel tiles (C = T*P = 512)
CH = 8    # column chunks (N = CH*S = 4096)
S = 512
C = 512
N = 4096

WVS = 32.0  # host pre-scale on Wv^T
MSC = 64.0  # host pre-scale on M^T (fp8 subnormal avoidance)

DR = mybir.MatmulPerfMode.DoubleRow


def round_f32r(a):
    v = np.ascontiguousarray(np.asarray(a, np.float32)).view(np.uint32)
    vr = (v + 0x1000 + ((v >> 13) & 1)) & 0xFFFFE000
    return vr.view(np.float32)


def build(gamma=0.5, has_bv=False, reps=None, no_xdma=False):
    nc = bacc.Bacc("TRN2", target_bir_lowering=False, debug=False)
    xq_d = nc.dram_tensor("xq", [CH * C, 2 * S], fp8, kind="ExternalInput")
    mq_d = nc.dram_tensor("mq", [C, 2 * C], fp8, kind="ExternalInput")
    wvs_d = nc.dram_tensor("wvs", [C, C], f32, kind="ExternalInput")   # 32*Wv^T
    gsc_d = nc.dram_tensor("gsc", [P, 1], f32, kind="ExternalInput")   # gamma/32
    gbv_d = nc.dram_tensor("gbv", [P, T], f32, kind="ExternalInput")   # gamma*bv
    y_d = nc.dram_tensor("y", [C, N], f32, kind="ExternalOutput")

    Exp = mybir.ActivationFunctionType.Exp
    Copy = mybir.ActivationFunctionType.Copy
    mult = mybir.AluOpType.mult
    sub_ = mybir.AluOpType.subtract
    add_ = mybir.AluOpType.add
    maxop = mybir.AluOpType.max
    AX = mybir.AxisListType.X

    with tile.TileContext(nc) as tc:
        with (
            tc.tile_pool(name="consts", bufs=1) as consts,
            tc.tile_pool(name="stage", bufs=1) as stagep,
            tc.tile_pool(name="hk", bufs=2) as hkp,
            tc.tile_pool(name="vkt", bufs=4) as vktp,
            tc.tile_pool(name="yout", bufs=2) as youtp,
            tc.tile_pool(name="pse", bufs=4, space="PSUM") as pse,
            tc.tile_pool(name="psv", bufs=2, space="PSUM") as psv,
            tc.tile_pool(name="pso", bufs=2, space="PSUM") as pso,
        ):
            # --- startup DMAs: x chunk 0 (SP) and M^T column slabs (Pool)
            # first -- these gate the first Hk matmuls.  Everything else
            # trickles in behind them.
            mq = consts.tile([P, T, 2, S], fp8, name="mq", tag="mq")
            xq = consts.tile([P, CH, T, 2, S], fp8, name="xq", tag="xq")
            if not no_xdma:
                nc.sync.dma_start(
                    out=xq[:, 0, :, :, :],
                    in_=xq_d[0:C, :].rearrange("(t p) c -> p t c", p=P),
                )
            else:
                nc.gpsimd.memset(xq[:, :, :, :, :], 0.25)
            nc.gpsimd.dma_start(
                out=mq[:, :, :, :],
                in_=mq_d[:, :].rearrange("(t p) c -> p t c", p=P),
            )
            gsc = consts.tile([P, 1], f32)
            nc.scalar.dma_start(out=gsc, in_=gsc_d[:, :])
            gbv = None
            if has_bv:
                gbv = consts.tile([P, T], f32, name="gbv", tag="gbv")
                nc.scalar.dma_start(out=gbv, in_=gbv_d[:, :])

            wvsb = stagep.tile([P, T, S], f32, name="wvsb", tag="wvsb")
            wq = consts.tile([P, T, 2, S], fp8, name="wq", tag="wq")
            aq = consts.tile([P, T, 2, S], fp8, name="aq", tag="aq")
            attnb = consts.tile([P, T, S], bf16, name="attnb", tag="attnb")
            identb = consts.tile([P, P], bf16)
            ii8 = consts.tile([P, 2, P], fp8, name="ii8", tag="ii8")
            negmax = consts.tile([P, T], f32)
            negmax64 = consts.tile([P, T], f32, name="negmax64", tag="negmax64")
            sums = consts.tile([P, T], f32)
            rsum = consts.tile([P, T], f32)

            import contextlib
            loop_ctx = tc.For_i(0, reps, 1) if reps else contextlib.nullcontext()
            loop_ctx.__enter__()

            en = [pse.tile([P, S], f32, name=f"en{i}", tag="energy") for i in range(T)]

            # ---------------- phase B: energy = sum_k Xk^T (M Xk) ----------------
            TERMS = [(0, 0), (0, 1), (1, 0)]  # (hi/lo, hi/lo) compensation
            for k in range(CH):
                sl = slice(S * k, S * (k + 1))
                if k > 0 and not no_xdma:
                    # k=1,2 on SP right behind chunk 0; later chunks on Pool
                    eng = nc.sync if k <= 2 else nc.gpsimd
                    eng.dma_start(
                        out=xq[:, k, :, :, :],
                        in_=xq_d[k * C:(k + 1) * C, :].rearrange("(t p) c -> p t c", p=P),
                    )
                if k == 3:
                    # Wv^T staging deferred off the startup critical path
                    nc.scalar.dma_start(
                        out=wvsb[:, :, :],
                        in_=wvs_d[:, :].rearrange("(t p) c -> p t c", p=P),
                    )
                if k == 4:
                    # wq: hi = fp8(wvs), lo = fp8(wvs - hi)
                    nc.scalar.copy(wq[:, :, 0, :], wvsb[:, :, :])
                    nc.vector.tensor_tensor(
                        out=wq[:, :, 1, :], in0=wvsb[:, :, :],
                        in1=wq[:, :, 0, :], op=sub_,
                    )

                # Hk = (64 M) Xk, 3-term fp8 DR; requant to hi/lo fp8
                hq = hkp.tile([P, T, 2, S], fp8, tag="hk", name="hq")
                for c1 in range(T):
                    hk_ps = psv.tile([P, S], f32, tag="vps", name="hk_ps")
                    i = 0
                    nmm = len(TERMS) * (T // 2)
                    for mi, xi in TERMS:
                        for cp in range(T // 2):
                            pr = slice(2 * cp, 2 * cp + 2)
                            nc.tensor.matmul(
                                hk_ps,
                                mq[:, pr, mi, P * c1:P * (c1 + 1)],
                                xq[:, k, pr, xi, :],
                                start=(i == 0),
                                stop=(i == nmm - 1),
                                perf_mode=DR,
                            )
                            i += 1
                    nc.scalar.copy(hq[:, c1, 0, :], hk_ps)
                    nc.vector.tensor_tensor(
                        out=hq[:, c1, 1, :], in0=hk_ps, in1=hq[:, c1, 0, :], op=sub_
                    )

                # en += Xk^T Hk, 3-term fp8 DR (energies carry a 64x scale)
                for si in range(T):
                    i = 0
                    for xi, hi in TERMS:
                        for cp in range(T // 2):
                            pr = slice(2 * cp, 2 * cp + 2)
                            nc.tensor.matmul(
                                en[si],
                                xq[:, k, pr, xi, P * si:P * (si + 1)],
                                hq[:, pr, hi, :],
                                start=(k == 0 and i == 0),
                                stop=(k == CH - 1 and i == len(TERMS) * (T // 2) - 1),
                                skip_group_check=True,
                                perf_mode=DR,
                            )
                            i += 1

            # identity consts built late so Pool services the mtr/x DMAs first
            make_identity(nc, identb)
            nc.vector.tensor_copy(ii8[:, 0, :], identb)
            nc.vector.tensor_copy(ii8[:, 1, :], identb)

            # ---------------- softmax (bf16 out), pipelined per-si ----------------
            for si in range(T):
                nc.vector.tensor_reduce(
                    out=negmax[:, si:si + 1], in_=en[si], axis=AX, op=maxop, negate=True
                )
            # negmax is in the 64x-scaled energy domain; exp() applies the
            # 1/64 rescale via its immediate scale on both input and bias
            nc.scalar.activation(
                out=negmax64[:, :], in_=negmax[:, :], func=Copy, scale=1.0 / MSC,
            )
            for si in range(T):
                nc.scalar.activation(
                    out=attnb[:, si, :],
                    in_=en[si],
                    func=Exp,
                    bias=negmax64[:, si:si + 1],
                    scale=1.0 / MSC,
                    accum_out=sums[:, si:si + 1],
                )
            for si in range(T):
                nc.vector.reciprocal(out=rsum[:, si:si + 1], in_=sums[:, si:si + 1])
                nc.vector.tensor_scalar_mul(
                    attnb[:, si, :], attnb[:, si, :], rsum[:, si:si + 1]
                )

            # ---------------- phase E ----------------
            def e_front(k):
                """V1T_k = Xk^T (32 Wv^T), 3-term fp8 DR; requant to hi/lo fp8
                with gamma/32 folded in."""
                vq = vktp.tile([P, T, 2, S], fp8, name="vq", tag="vq")
                for ms in range(T):
                    v_ps = psv.tile([P, S], f32, tag="vps", name="v_ps")
                    blk = slice(P * ms, P * (ms + 1))
                    i = 0
                    terms = [(0, 0), (0, 1), (1, 0)]  # (x hi/lo, w hi/lo)
                    nmm = len(terms) * (T // 2)
                    for xi, wi in terms:
                        for cp in range(T // 2):
                            pr = slice(2 * cp, 2 * cp + 2)
                            nc.tensor.matmul(
                                v_ps,
                                xq[:, k, pr, xi, blk],
                                wq[:, pr, wi, :],
                                start=(i == 0),
                                stop=(i == nmm - 1),
                                perf_mode=DR,
                            )
                            i += 1
                    # hi on ACT (immediate scale), lo on DVE (scalar AP)
                    nc.scalar.activation(
                        out=vq[:, ms, 0, :], in_=v_ps, func=Copy,
                        scale=float(gamma) / WVS,
                    )
                    nc.vector.scalar_tensor_tensor(
                        out=vq[:, ms, 1, :], in0=v_ps, scalar=gsc[:, 0:1],
                        in1=vq[:, ms, 0, :], op0=mult, op1=sub_,
                    )
                return vq

            def e_back(k, vq):
                sl = slice(S * k, S * (k + 1))
                yo = youtp.tile([P, T, S], f32, tag="yo", name="yo")
                for os in range(T):
                    o_ps = pso.tile([P, S], f32, tag="ops", name="o_ps")
                    i = 0
                    terms = [(0, 0), (0, 1), (1, 0)]  # (v hi/lo, a hi/lo)
                    for vi, ai in terms:
                        for jp in range(T // 2):
                            pr = slice(2 * jp, 2 * jp + 2)
                            nc.tensor.matmul(
                                o_ps,
                                vq[:, pr, vi, P * os:P * (os + 1)],
                                aq[:, pr, ai, :],
                                start=(i == 0),
                                stop=False,
                                perf_mode=DR,
                            )
                            i += 1
                    # residual: + I.(x8+xe8) rides through the PE, so the
                    # PSUM result IS y and the copy-out is a plain copy
                    nc.tensor.matmul(
                        o_ps,
                        ii8[:, 0:2, :],
                        xq[:, k, os, 0:2, :],
                        start=False,
                        stop=True,
                        perf_mode=DR,
                    )
                    if has_bv:
                        nc.vector.tensor_scalar(
                            out=o_ps, in0=o_ps, scalar1=gbv[:, os:os + 1], op0=add_
                        )
                    if os % 2 == 0:
                        nc.scalar.copy(yo[:, os, :], o_ps)
                    else:
                        nc.vector.tensor_copy(yo[:, os, :], o_ps)
                    if not no_xdma and k >= CH - 1:
                        # tail: per-tile stores on distinct queues so their
                        # DGE latencies overlap
                        ydma = [nc.sync, nc.scalar, nc.gpsimd, nc.sync]
                        ydma[os].dma_start(
                            out=y_d[P * os:P * (os + 1), sl], in_=yo[:, os, :]
                        )
                if not no_xdma and k < CH - 1:
                    dma_engs = [nc.sync, nc.scalar]
                    dma_engs[k % 2].dma_start(
                        out=y_d[:, sl].rearrange("(t p) c -> p t c", p=P),
                        in_=yo[:, :, :],
                    )

            lookahead = 3
            pending = [e_front(i) for i in range(lookahead)]

            # attn^T: bf16 PE transposes written into the DEAD en[si] psum
            # tiles (bitcast views; WAR on exp(si) is tracked), hi/lo fp8
            # quantize on the way out
            for si in range(T):
                for jt in range(T):
                    trp = en[si][:, 64 * jt:64 * (jt + 1)].bitcast(bf16)
                    nc.tensor.transpose(trp, attnb[:, si, P * jt:P * (jt + 1)], identb)
                    cs = slice(P * si, P * (si + 1))
                    nc.scalar.copy(aq[:, jt, 0, cs], trp)
                    nc.vector.tensor_tensor(
                        out=aq[:, jt, 1, cs], in0=trp, in1=aq[:, jt, 0, cs], op=sub_
                    )

            for k in range(CH):
                if k + lookahead < CH:
                    pending.append(e_front(k + lookahead))
                vq = pending.pop(0)
                e_back(k, vq)

            loop_ctx.__exit__(None, None, None)

    nc.compile()
    return nc


_NC_CACHE = {}


def _get_nc(gamma=0.5, has_bv=False):
    key = (float(gamma), has_bv)
    if key not in _NC_CACHE:
        _NC_CACHE[key] = build(gamma, has_bv)
    return _NC_CACHE[key]


def prep_in_maps(x, Wq, bq, Wk, bk, Wv, bv, gamma):
    x = np.ascontiguousarray(np.asarray(x, np.float32))
    B = x.shape[0]
    assert x.shape == (B, C, 64, 64) and B == 8, x.shape
    if np.any(np.asarray(bq)) or np.any(np.asarray(bk)):
        raise NotImplementedError("nonzero q/k biases not supported")

    import ml_dtypes
    e4 = ml_dtypes.float8_e4m3

    def q8pair(a):
        hi = np.asarray(a, np.float32).astype(e4)
        lo = (np.asarray(a, np.float32) - hi.astype(np.float32)).astype(e4)
        return hi, lo

    def rowcat(hi, lo):
        # [R, F] hi/lo -> [R, 2F] with per-row [hi | lo]
        return np.ascontiguousarray(
            np.concatenate([hi[:, None, :], lo[:, None, :]], axis=1).reshape(
                hi.shape[0], -1)
        )

    def xq_arr(xb):
        # (C, N) -> chunk-major [(CH C), 2 S]: per chunk, per channel [hi|lo]
        hi, lo = q8pair(xb)
        hi = hi.reshape(C, CH, S).transpose(1, 0, 2)
        lo = lo.reshape(C, CH, S).transpose(1, 0, 2)
        out = np.concatenate([hi[:, :, None, :], lo[:, :, None, :]], axis=2)
        return np.ascontiguousarray(out.reshape(CH * C, 2 * S))

    mts = (np.asarray(Wk, np.float64).T @ np.asarray(Wq, np.float64)).astype(
        np.float32) * np.float32(MSC)
    mq = rowcat(*q8pair(mts))
    wvs = np.ascontiguousarray(np.asarray(Wv, np.float32).T * np.float32(WVS))
    g = np.float32(np.asarray(gamma).reshape(-1)[0])
    gsc = np.full((P, 1), g / np.float32(WVS), np.float32)
    gbv = np.ascontiguousarray(
        (g * np.asarray(bv, np.float32)).reshape(T, P).T
    ).astype(np.float32)

    return [
        {
            "xq": xq_arr(x[b].reshape(C, N)),
            "mq": mq,
            "wvs": wvs,
            "gsc": gsc,
            "gbv": gbv,
        }
        for b in range(B)
    ]


def kernel(x, Wq, bq, Wk, bk, Wv, bv, gamma):
    in_maps = prep_in_maps(x, Wq, bq, Wk, bk, Wv, bv, gamma)
    g = float(np.asarray(gamma).reshape(-1)[0])
    has_bv = bool(np.any(np.asarray(bv)))
    nc = _get_nc(g, has_bv)
    res = run_bass_kernel_spmd(nc, in_maps, core_ids=list(range(8)))
    out = np.stack([res.results[b]["y"].reshape(C, 64, 64) for b in range(8)])
    return out.astype(np.float32)
